# revision 15
# baseline (speedup 1.0000x reference)
"""Trainium2 Bass kernel for nn_JSDPosLoss: JSD loss over top-k retrieved rows.

Contract: kernel(**inputs) takes FULL numpy inputs, returns FULL output (f32
scalar). Data-parallel over batch across 8 NeuronCores (4 batches/core).

v6 design (memory-regime): stream z_pos as fp8e4m3 (4 MiB/core; attention
only ranks top-k and the loss is insensitive to rank flips) into DoubleRow
fp8 matmuls (256-deep contraction per pass). Zero-padded lhsT tiles place
each batch's 3 query rows and all 8 column chunks at distinct partitions of
one [64, 512] PSUM tile, so top-k passes cost only 512 columns and there
are no PSUM->SBUF copies.

Per batch, software-pipelined against the remaining stream:
  max8 + max_index (one 512-col pass each) -> pack quantized value + column
  index into one f32 -> tiny f32 PE matmuls fold candidates across
  partitions into [3, 64] -> max8/match_replace/max8 merge to top-10 ->
  unpack indices (mod 8192) straight to u32 [3, 10], used directly as a 2D
  indirect-DMA offset AP (no flatten):
    gather 1: -g rows (host-negated bf16 table) accumulated via the DMA
      compute-op onto -p prefilled tiles -> -s with zero engine work;
    gather 2: host-precomputed f32 row sums of g*ln(g) -> [30, 1].
  JSD partial: acc = sum((-s) * ln(-s * -0.5)) + gathered-gsum, computed by
  one Ln + one tensor_tensor_reduce per batch pair (gsum rides in as the
  reduction's per-partition initial value). Host adds sum(xlogy(p, p)).

Tiles are per batch-pair so gather/compute chains of different pairs never
serialize on tile-granularity hazards. Pair rows padded 30->32 per batch so
every engine slice starts at a 0/32/64/96 partition base.
"""

import numpy as np
import ml_dtypes

import concourse.bass as bass
import concourse.bacc as bacc
import concourse.mybir as mybir
import concourse.tile as tile
from concourse.bass_utils import run_bass_kernel_spmd

# Problem dims (hardcoded per contract)
B, H, W, D, NPQ = 32, 64, 64, 256, 512
HW = H * W                  # 4096
NQ, NPOS = 3, 10
NCORES = 8
BPC = B // NCORES           # 4 batches per core
NPR = 32                    # padded pair-rows per batch (30 used)
NPAD = BPC * NPR            # 128 padded pair rows per core

F32 = mybir.dt.float32
BF16 = mybir.dt.bfloat16
F8 = mybir.dt.float8e4
U32 = mybir.dt.uint32
AF = mybir.ActivationFunctionType
ALU = mybir.AluOpType
JDT = BF16                  # JSD elementwise dtype (accumulation is f32)

NCH = 8                     # column chunks per batch row
CW = HW // NCH              # 512 columns per chunk
MAGIC = 12582912.0          # 1.5 * 2**23: float32 round-to-int trick
QS = 16.0                   # value quantization scale for packing
PKS = 8192.0                # index field size in packed floats


def build_kernel():
    nc = bacc.Bacc("TRN2", target_bir_lowering=False, debug=False,
                   num_devices=NCORES)

    # z_pos fp8, DoubleRow layout: [bi, c(part), kt, j]; d = 128*kt + c
    zpt = nc.dram_tensor("zpt", [BPC, 128, 2, HW], F8,
                         kind="ExternalInput").ap()
    # zero-padded stationary tiles [c, bi, ch, kt, m]: chunk ch's queries
    # at out-partition m = 32*(ch//4) + 3*(ch%4) + q, full 64-wide dst
    lw = nc.dram_tensor("lw", [128, BPC, NCH, 2, 64], F8,
                        kind="ExternalInput").ap()
    # gather tables: negated bf16 z_pos_dis rows + f32 row sums of g*ln(g)
    zpdneg = nc.dram_tensor("zpdneg", [BPC * HW, NPQ], BF16,
                            kind="ExternalInput").ap()
    gsum = nc.dram_tensor("gsum", [BPC * HW, 1], F32,
                          kind="ExternalInput").ap()
    # negated P rows (-sample_z_dis broadcast, torch-quirk order), padded
    pmn = nc.dram_tensor("pmn", [NPAD, NPQ], BF16, kind="ExternalInput").ap()
    # fold selectors [64, 3*NCH] f32 + per-partition chunk column offsets
    selc = nc.dram_tensor("selc", [64, 3 * NCH], F32,
                          kind="ExternalInput").ap()
    offc = nc.dram_tensor("offc", [64, 1], F32, kind="ExternalInput").ap()
    # output per padded pair-row: gsum + sum((-s) ln(s/2))
    out = nc.dram_tensor("out", [NPAD, 1], F32, kind="ExternalOutput").ap()

    with tile.TileContext(nc) as tc:
        _body(tc, nc, zpt, lw, zpdneg, gsum, pmn, selc, offc, out)
    nc.compile()
    return nc


def _body(tc, nc, zpt, lw, zpdneg, gsum, pmn, selc, offc, out):
    with (
        tc.tile_pool(name="const", bufs=1) as cpool,
        tc.tile_pool(name="load", bufs=2) as lpool,
        tc.tile_pool(name="attn", bufs=2, space="PSUM") as apool,
        tc.tile_pool(name="fold", bufs=2, space="PSUM") as fpool,
        tc.tile_pool(name="small", bufs=2) as spool,
    ):
        # ---- constants / startup (st prefills early: transfers are tiny
        # and the add-gathers depend on them) ----
        lw_sb = cpool.tile([128, BPC, NCH, 2, 64], F8)
        nc.sync.dma_start(lw_sb[:], lw[:])
        st = []      # per-pair [64, NPQ]: -p prefilled, gathers add -g -> -s
        gsm = []     # per-pair [64, 1] f32 gathered row sums of g*ln(g)
        lm = []      # per-pair ln(s/2)
        for p in range(2):
            stp = cpool.tile([64, NPQ], JDT, name=f"st{p}")
            nc.sync.dma_start(stp[:], pmn[64 * p:64 * p + 64, :])
            st.append(stp)
            g = cpool.tile([64, 1], F32, name=f"gsm{p}")
            nc.vector.memset(g[:], 0.0)
            gsm.append(g)
            lm.append(cpool.tile([64, NPQ], JDT, name=f"lm{p}"))
        tt = cpool.tile([64, NPQ], JDT)
        acc = cpool.tile([NPAD, 1], F32)
        sel_sb = cpool.tile([64, 3 * NCH], F32)
        nc.gpsimd.dma_start(sel_sb[:], selc[:, :])
        off_sb = cpool.tile([64, 1], F32)
        nc.gpsimd.dma_start(off_sb[:], offc[:, :])
        one = cpool.tile([32, 1], F32)
        nc.vector.memset(one[:], 1.0)
        bias38 = cpool.tile([NPAD, 1], F32)
        nc.vector.memset(bias38[:], 1e-38)

        lds = {}

        # stream segmentation per batch: b0/b3 quartered (early pipeline
        # start; tail matmuls drain right behind the stream)
        SEGS = {0: 4, 1: 1, 2: 1, 3: 4}

        def stage_stream(bi, engs):
            n = SEGS[bi]
            w = HW // n
            segs = []
            for t in range(n):
                ld = lpool.tile([128, 2, w], F8, tag=f"ld{bi}_{t}")
                engs[t % len(engs)].dma_start(
                    ld[:], zpt[bi, :, :, t * w:(t + 1) * w])
                segs.append(ld)
            lds[bi] = (segs, w)

        def stage_attn(bi):
            # PSUM [64, 512]: partition p = 32*(ch//4) + 3*(ch%4) + q
            segs, w = lds[bi]
            at = apool.tile([64, CW], F32, tag="at")
            for ch in range(NCH):
                base = ch * CW
                rhs = segs[base // w][:, :, base % w:base % w + CW]
                nc.tensor.matmul(
                    at[:, :],
                    lhsT=lw_sb[:, bi, ch],
                    rhs=rhs,
                    start=(ch == 0), stop=(ch == NCH - 1),
                    perf_mode=mybir.MatmulPerfMode.DoubleRow)
            return at

        def stage_select(bi, at):
            # top-8 per chunk (value + index), one 512-col pass each
            cand = spool.tile([64, 8], F32, tag="cand")
            candi = spool.tile([64, 8], U32, tag="candi")
            nc.vector.max(cand[:], at[:])
            nc.vector.max_index(candi[:], cand[:], at[:])

            # pack quantized value + in-batch column index into one f32
            idxf = spool.tile([64, 8], F32, tag="idxf")
            nc.vector.tensor_scalar(idxf[:], candi[:], off_sb[:], None,
                                    op0=ALU.add)
            pk = spool.tile([64, 8], F32, tag="pk")
            nc.vector.tensor_scalar(pk[:], cand[:], QS, MAGIC,
                                    op0=ALU.mult, op1=ALU.add)
            nc.vector.tensor_scalar(pk[:], pk[:], PKS, MAGIC * PKS,
                                    op0=ALU.mult, op1=ALU.subtract)
            nc.vector.tensor_add(pk[:], pk[:], idxf[:])

            # fold candidates across partitions: [64, 8] -> [3, 64]
            fp = fpool.tile([NQ, 8 * NCH], F32, tag="fp")
            for ch in range(NCH):
                nc.tensor.matmul(
                    fp[:, 8 * ch:8 * ch + 8],
                    lhsT=sel_sb[:, 3 * ch:3 * ch + NQ],
                    rhs=pk[:],
                    start=True, stop=True)

            # merge to top-10 packed per row
            m1 = spool.tile([NQ, 8], F32, tag="m1")
            nc.vector.max(m1[:], fp[:])
            tmp = spool.tile([NQ, 8 * NCH], F32, tag="tmp")
            nc.vector.match_replace(tmp[:], in_to_replace=m1[:],
                                    in_values=fp[:], imm_value=-1e30)
            m2 = spool.tile([NQ, 8], F32, tag="m2")
            nc.vector.max(m2[:], tmp[:])

            # unpack row index (+ batch base) straight to u32, [3, 10]
            i10 = spool.tile([NQ, NPOS], U32, tag="i10")
            nc.vector.tensor_scalar(i10[:, 0:8], m1[:], PKS, float(bi * HW),
                                    op0=ALU.mod, op1=ALU.add)
            nc.vector.tensor_scalar(i10[:, 8:NPOS], m2[:, 0:2], PKS,
                                    float(bi * HW),
                                    op0=ALU.mod, op1=ALU.add)

            # gathers (2D offset AP): -g accumulated onto -p -> -s, and the
            # precomputed per-row g*ln(g) sums
            p, u = bi // 2, bi % 2
            rr = slice(NPR * u, NPR * u + NQ * NPOS)
            nc.gpsimd.indirect_dma_start(
                out=st[p][rr, :], out_offset=None,
                in_=zpdneg[:, :],
                in_offset=bass.IndirectOffsetOnAxis(ap=i10[:, :], axis=0),
                compute_op=ALU.add)
            nc.gpsimd.indirect_dma_start(
                out=gsm[p][rr, :], out_offset=None,
                in_=gsum[:, :],
                in_offset=bass.IndirectOffsetOnAxis(ap=i10[:, :], axis=0))

        def stage_jsd(p):
            # lm = ln(-s * -0.5); acc = gsum + sum((-s) * lm) per row
            nc.scalar.activation(lm[p][:], st[p][:], AF.Ln,
                                 bias=bias38[0:64], scale=-0.5)
            nc.vector.tensor_tensor_reduce(
                out=tt[:], in0=st[p][:], in1=lm[p][:],
                scale=1.0, scalar=gsm[p][:],
                op0=ALU.mult, op1=ALU.add,
                accum_out=acc[64 * p:64 * p + 64, :])

        # ---- software-pipelined emission ----
        stage_stream(0, [nc.sync, nc.scalar])
        stage_stream(1, [nc.sync])
        at0 = stage_attn(0)
        at1 = stage_attn(1)
        # preload the Ln activation table off the critical path
        nc.scalar.activation(one[:], one[:], AF.Ln, bias=bias38[0:32])
        stage_select(0, at0)
        stage_stream(2, [nc.scalar])
        stage_select(1, at1)
        at2 = stage_attn(2)
        stage_stream(3, [nc.sync, nc.scalar])
        stage_select(2, at2)
        stage_jsd(0)
        at3 = stage_attn(3)
        stage_select(3, at3)
        stage_jsd(1)

        nc.sync.dma_start(out[:, :], acc[:])


_CACHE = {}


def _prep_in_maps(z, z_pos, z_dis, z_pos_dis, rand_idx):
    f8 = ml_dtypes.float8_e4m3
    bf = ml_dtypes.bfloat16
    zf = z.reshape(B, HW, D)
    zpdf = z_pos_dis.reshape(B, HW, NPQ).astype(np.float32, copy=False)
    zposf = z_pos.reshape(B, HW, D).astype(np.float32, copy=False)
    zdf = z_dis.reshape(B, HW, NPQ)

    ridx = rand_idx.astype(np.int64)
    sample_z = np.take_along_axis(zf, ridx[..., None], axis=1)       # (B,3,D)
    sample_z_dis = np.take_along_axis(zdf, ridx[..., None], axis=1)  # (B,3,NPQ)

    # fold selectors / chunk offsets (shared across cores)
    selc = np.zeros((64, 3 * NCH), np.float32)
    offc = np.zeros((64, 1), np.float32)
    for ch in range(NCH):
        for q in range(NQ):
            p = 32 * (ch // 4) + 3 * (ch % 4) + q
            selc[p, 3 * ch + q] = 1.0
            offc[p, 0] = CW * ch

    jmod = np.arange(NQ * NPOS) % NQ

    in_maps = []
    for c in range(NCORES):
        bs = slice(c * BPC, (c + 1) * BPC)
        # zpt[bi, c, kt, j] = z_pos[4core+bi, j, 128*kt+c]
        zpt = np.ascontiguousarray(
            zposf[bs].transpose(0, 2, 1).reshape(BPC, 2, 128, HW)
            .transpose(0, 2, 1, 3)).astype(f8)
        # lw[c, bi, v, kt, m]: batch bi queries at m = 3v+q
        sz8 = sample_z[bs].astype(f8)                      # (BPC, 3, D)
        szt = np.ascontiguousarray(
            sz8.reshape(BPC, NQ, 2, 128).transpose(3, 0, 2, 1))  # c,bi,kt,q
        lwf = np.zeros((128, BPC, NCH, 2, 64), f8)
        for ch in range(NCH):
            m = 32 * (ch // 4) + 3 * (ch % 4)
            lwf[:, :, ch, :, m:m + NQ] = szt
        # negated P rows, padded: row 32*bi + j = -sample_z_dis[., j % 3]
        szd = sample_z_dis[bs].astype(np.float32)          # (BPC, 3, NPQ)
        pmn = np.zeros((BPC, NPR, NPQ), np.float32)
        pmn[:, :NQ * NPOS] = -szd[:, jmod, :]
        # bf16 g rows as gathered by the device; gsum from those bf16 values
        # so device and host terms cancel consistently
        g_bf = zpdf[bs].reshape(BPC * HW, NPQ).astype(bf)
        g64 = g_bf.astype(np.float64)
        gsum = np.where(g64 > 0, g64 * np.log(np.where(g64 > 0, g64, 1.0)),
                        0.0).sum(axis=1, keepdims=True).astype(np.float32)
        in_maps.append({
            "zpt": zpt,
            "lw": lwf,
            "zpdneg": -g_bf,
            "gsum": gsum,
            "pmn": pmn.reshape(NPAD, NPQ).astype(bf),
            "selc": selc,
            "offc": offc,
        })
    return in_maps


def kernel(z, z_pos, z_dis, z_pos_dis, rand_idx):
    if "nc" not in _CACHE:
        _CACHE["nc"] = build_kernel()
    nc = _CACHE["nc"]
    in_maps = _prep_in_maps(z, z_pos, z_dis, z_pos_dis, rand_idx)
    res = run_bass_kernel_spmd(nc, in_maps, core_ids=list(range(NCORES)))

    # host: sum(xlogy(p,p)) + per-row accumulator; skip pad rows
    valid = (np.arange(NPAD) % NPR) < NQ * NPOS
    total = 0.0
    for c in range(NCORES):
        o = res.results[c]["out"].astype(np.float64)[valid]
        total += o.sum()
        p = -in_maps[c]["pmn"].astype(np.float64)[valid]
        total += np.where(p > 0, p * np.log(np.where(p > 0, p, 1.0)), 0.0).sum()
    loss = 0.5 * total / (B * NQ * NPOS)
    return np.float32(loss)
